# revision 13
# baseline (speedup 1.0000x reference)
"""Trainium2 Bass kernel for cross "efficient attention".

Reference computation (per batch b, head h, with C=128, HEADS=8, hc=16, n=16384):
    k = x2[b].reshape(HEADS, hc, n); v = x1[b].reshape(HEADS, hc, n)
    key_sm   = softmax(k, axis=-1)          # over n
    query_sm = softmax(k, axis=1)           # over hc (head channels)
    context  = key_sm @ v^T                 # (hc, hc)
    out[b,h] = context^T @ query_sm         # (hc, n)

Sharding: data-parallel over batch B=8 across the 8 NeuronCores (no
collectives).  Inputs are ~N(0,1) so softmax needs no max-subtraction.

Layout: the host pre-transposes BOTH inputs into an "A-layout"
[128, N] bf16 array (partition p holds n = j*128+p for every
channel-block j), so every DMA is a plain contiguous slice.  x1 gets a
ones-column per block (129 channels) so the context matmul emits
rowsums for free.  Total HBM traffic 12 MiB/core (8 in + 4 out) —
the kernel is structured to be bound by exactly that.

Single merged pipeline per slab of n (no phase barrier):
    eT   = exp(x2T)                                    (scalar)
    ctx += eT_j^T @ [vT_j | 1]                         (PE, early: only
                                                        needs eT, so bd is
                                                        ready right after
                                                        the last input)
    csT  = per-head colsums: pairwise tree             (gpsimd 8-wide step,
                                                        vector small steps)
    qsmT = eT * bcast(1/csT)    (pair-packed, DVE 2x)  (vector)
    qsm  = PE-transpose(qsmT), lagged 2 slabs          (PE; PSUM->SBUF
                                                        copies on vector)
Then bd = blockdiag(ctx * 1/rowsum) and the attended tiles
    out[:, t] = bd^T @ qsm[:, t]                       (PE + casts + DMA)
stream out per-tile as soon as bd and their qsm window exist.
"""

import numpy as np
from contextlib import ExitStack

B, C, H, W = 8, 128, 128, 128
N = H * W                 # 16384
J = N // 128              # 128 channel-blocks
HEADS, HC = 8, 16
NCORES = 8

SLABS = [2048] * 7 + [1024, 512, 512]
assert sum(SLABS) == N
LAG = 2                   # transpose lag (slabs) so the PE never stalls
QS = 512                  # tail matmul moving-operand tile (one PSUM bank f32)
OT = 2048                 # tail output tile width
NOT = N // OT             # 8

_cache: dict = {}


def _build():
    import concourse.bass as bass
    import concourse.tile as tile
    from concourse import bacc, mybir

    FP32 = mybir.dt.float32
    BF16 = mybir.dt.bfloat16
    AF = mybir.ActivationFunctionType

    nc = bacc.Bacc("TRN2", target_bir_lowering=False, debug=False)

    x2p = nc.dram_tensor("x2p", [128, N], BF16, kind="ExternalInput")
    x1p = nc.dram_tensor("x1p", [128, J * 129], BF16, kind="ExternalInput")
    bd8_in = nc.dram_tensor("bd8", [C, C], BF16, kind="ExternalInput")
    ident_in = nc.dram_tensor("ident", [C, C], BF16, kind="ExternalInput")
    out = nc.dram_tensor("out", [C, N], BF16, kind="ExternalOutput")

    with tile.TileContext(nc) as tc:
        with ExitStack() as ctx:
            persist = ctx.enter_context(tc.tile_pool(name="persist", bufs=1))
            x2ld = ctx.enter_context(tc.tile_pool(name="x2ld", bufs=3))
            vld = ctx.enter_context(tc.tile_pool(name="vld", bufs=3))
            eTp = ctx.enter_context(tc.tile_pool(name="eTp", bufs=3))
            qsp = ctx.enter_context(tc.tile_pool(name="qsp", bufs=4))
            nrm = ctx.enter_context(tc.tile_pool(name="nrm", bufs=3))
            outp = ctx.enter_context(tc.tile_pool(name="outp", bufs=3))
            smalls = ctx.enter_context(tc.tile_pool(name="smalls", bufs=1))

            qsm_nat = persist.tile([C, N], BF16, tag="qsm_nat")
            bd8 = smalls.tile([C, C], BF16, tag="bd8")
            ident = smalls.tile([C, C], BF16, tag="ident")

            with tc.tile_pool(name="pstr", bufs=3, space="PSUM") as ps_tr, \
                 tc.tile_pool(name="psctx", bufs=1, space="PSUM") as ps_ctx:
                ctx_ps = ps_ctx.tile([C, 129], FP32, tag="ctx")

                mm_idx = 0
                trq = []          # queued (qsmT, off, SW) awaiting transpose

                def emit_transpose(qsmT, toff, SW, copy_eng):
                    te = ps_tr.tile([C, SW], BF16, tag="te")
                    for j in range(SW // 128):
                        nc.tensor.transpose(
                            te[:, bass.ts(j, 128)],
                            qsmT[:, bass.ds(j * 128, 128)],
                            ident[:],
                        )
                    go = bass.ds(toff, SW)
                    if copy_eng == "v":
                        nc.vector.tensor_copy(qsm_nat[:, go], te[:])
                    else:
                        nc.scalar.copy(qsm_nat[:, go], te[:])

                off = 0
                for i, SW in enumerate(SLABS):
                    Ji = SW // 128
                    G = Ji * 8
                    x2t = x2ld.tile([128, SW], BF16, tag="x2t")
                    nc.sync.dma_start(out=x2t[:], in_=x2p[:, bass.ds(off, SW)])
                    vT = vld.tile([128, Ji * 129], BF16, tag="vT")
                    nc.sync.dma_start(
                        out=vT[:],
                        in_=x1p[:, bass.ds((off // 128) * 129, Ji * 129)],
                    )
                    if i == 0:
                        nc.scalar.dma_start(out=ident[:], in_=ident_in[:])
                        nc.scalar.dma_start(out=bd8[:], in_=bd8_in[:])

                    eT = eTp.tile([128, SW], BF16, tag="eT")
                    nc.scalar.activation(eT[:], x2t[:], AF.Exp)

                    # context accumulation: current slab, ahead of any
                    # transpose in PE order — its inputs are ready early.
                    for j in range(Ji):
                        nc.tensor.matmul(
                            ctx_ps[:],
                            eT[:, bass.ts(j, 128)],
                            vT[:, j * 129:(j + 1) * 129],
                            start=(mm_idx == 0),
                            stop=(mm_idx == J - 1),
                        )
                        mm_idx += 1

                    # per-head colsums: pairwise tree (bf16). The wide first
                    # step runs on gpsimd; the narrow rest on vector.
                    e4 = eT[:].rearrange("p (g c) -> p g c", c=16)
                    t1 = nrm.tile([128, G * 8], BF16, tag="t1")
                    with nc.allow_low_precision("bf16 pairwise colsum; tol 2e-2"):
                        nc.gpsimd.tensor_add(
                            t1[:].rearrange("p (g c) -> p g c", c=8),
                            e4[:, :, 0:8], e4[:, :, 8:16],
                        )
                        t2 = nrm.tile([128, G * 4], BF16, tag="t2")
                        t1v = t1[:].rearrange("p (g c) -> p g c", c=8)
                        nc.vector.tensor_add(
                            t2[:].rearrange("p (g c) -> p g c", c=4),
                            t1v[:, :, 0:4], t1v[:, :, 4:8],
                        )
                        t3 = nrm.tile([128, G * 2], BF16, tag="t3")
                        t2v = t2[:].rearrange("p (g c) -> p g c", c=4)
                        nc.vector.tensor_add(
                            t3[:].rearrange("p (g c) -> p g c", c=2),
                            t2v[:, :, 0:2], t2v[:, :, 2:4],
                        )
                        csf = nrm.tile([128, G], FP32, tag="csf")
                        t3v = t3[:].rearrange("p (g c) -> p g c", c=2)
                        nc.vector.tensor_add(
                            csf[:].rearrange("p (g o) -> p g o", o=1),
                            t3v[:, :, 0:1], t3v[:, :, 1:2],
                        )
                    rcf = nrm.tile([128, G], FP32, tag="rcf")
                    nc.vector.reciprocal_approx_fast(out=rcf[:], in_=csf[:])
                    # duplicate reciprocals into packed bf16 pairs so the qsm
                    # multiply keeps an innermost unit stride (DVE 2x mode)
                    rc2 = nrm.tile([128, G * 2], BF16, tag="rc2")
                    nc.gpsimd.tensor_copy(
                        rc2[:].rearrange("p (g t) -> p g t", t=2),
                        rcf[:, :, None].broadcast_to([128, G, 2]),
                    )
                    qsmT = qsp.tile([128, SW], BF16, tag="qsmT")
                    nc.vector.tensor_mul(
                        qsmT[:].rearrange("p (g a t) -> p g a t", a=8, t=2),
                        eT[:].rearrange("p (g a t) -> p g a t", a=8, t=2),
                        rc2[:].rearrange("p (g t) -> p g t", t=2)[:, :, None, :]
                              .broadcast_to([128, G, 8, 2]),
                    )
                    trq.append((qsmT, off, SW))
                    if len(trq) > LAG:
                        emit_transpose(*trq.pop(0), copy_eng="v" if i < 9 else "s")
                    off += SW
                for k, args in enumerate(trq):
                    emit_transpose(*args, copy_eng="s")

                # ---- block-diagonal context weights ----
                rs_rcp = smalls.tile([C, 1], FP32, tag="rs_rcp")
                nc.vector.reciprocal(rs_rcp[:], ctx_ps[:, 128:129])
                scaled = smalls.tile([C, C], BF16, tag="scaled")
                nc.vector.tensor_scalar(
                    scaled[:], ctx_ps[:, 0:128], rs_rcp[:, 0:1], None,
                    mybir.AluOpType.mult,
                )
                bd = smalls.tile([C, C], BF16, tag="bd")
                nc.vector.tensor_mul(bd[:], scaled[:], bd8[:])

            # ---- attended tiles: matmul, cast, store ----
            with tc.tile_pool(name="psatt", bufs=2, space="PSUM") as ps_att:
                for t in range(NOT):
                    att = ps_att.tile([C, OT], FP32, tag="att")
                    for q in range(OT // QS):
                        nc.tensor.matmul(
                            att[:, bass.ts(q, QS)], bd[:],
                            qsm_nat[:, bass.ds(t * OT + q * QS, QS)],
                        )
                    ot = outp.tile([C, OT], BF16, tag="ot")
                    if t in (1, 6):
                        nc.vector.tensor_copy(ot[:], att[:])
                    else:
                        nc.scalar.copy(ot[:], att[:])
                    nc.scalar.dma_start(out=out[:, bass.ts(t, OT)], in_=ot[:])

    nc.compile()
    return nc


def _get_nc():
    if "nc" not in _cache:
        _cache["nc"] = _build()
    return _cache["nc"]


def _bd8_np() -> np.ndarray:
    import ml_dtypes

    m = np.zeros((C, C), dtype=np.float32)
    for h in range(HEADS):
        m[h * HC:(h + 1) * HC, h * HC:(h + 1) * HC] = 1.0
    return m.astype(ml_dtypes.bfloat16)


def _ident_np() -> np.ndarray:
    import ml_dtypes

    return np.eye(C, dtype=np.float32).astype(ml_dtypes.bfloat16)


def _to_np(a) -> np.ndarray:
    """Materialize to float32 numpy; retry once on a transient bad fetch
    (device-backed arrays have been observed to materialize NaNs once)."""
    out = np.asarray(a, dtype=np.float32)
    if np.isnan(out).any():
        out = np.asarray(a, dtype=np.float32)
    return out


def _in_maps(x1: np.ndarray, x2: np.ndarray) -> list[dict]:
    """Host-side sharding + layout: per-core A-layout bf16 arrays."""
    import ml_dtypes

    BF = ml_dtypes.bfloat16
    x1 = _to_np(x1).reshape(B, C, N)
    x2 = _to_np(x2).reshape(B, C, N)
    # A-layout: arr[b, p, j, c] = x[b, c, j*128 + p]
    x2a = np.ascontiguousarray(
        x2.reshape(B, C, J, 128).transpose(0, 3, 2, 1)).astype(BF)
    x1a = x1.reshape(B, C, J, 128).transpose(0, 3, 2, 1).astype(BF)
    x1e = np.ones((B, 128, J, 129), dtype=BF)
    x1e[..., :128] = x1a
    x2a = x2a.reshape(B, 128, N)
    x1e = x1e.reshape(B, 128, J * 129)
    bd8 = _bd8_np()
    ident = _ident_np()
    return [
        {"x2p": x2a[i], "x1p": x1e[i], "bd8": bd8, "ident": ident}
        for i in range(NCORES)
    ]


def kernel(x1: np.ndarray, x2: np.ndarray) -> np.ndarray:
    from concourse.bass_utils import run_bass_kernel_spmd

    nc = _get_nc()
    in_maps = _in_maps(x1, x2)
    res = run_bass_kernel_spmd(nc, in_maps, core_ids=list(range(NCORES)))
    outs = [np.asarray(res.results[i]["out"], dtype=np.float32) for i in range(NCORES)]
    return np.stack(outs, axis=0).reshape(B, C, H, W)


# revision 17
# speedup vs baseline: 1.0687x; 1.0687x over previous
"""Trainium2 Bass kernel for cross "efficient attention".

Reference computation (per batch b, head h, with C=128, HEADS=8, hc=16, n=16384):
    k = x2[b].reshape(HEADS, hc, n); v = x1[b].reshape(HEADS, hc, n)
    key_sm   = softmax(k, axis=-1)          # over n
    query_sm = softmax(k, axis=1)           # over hc (head channels)
    context  = key_sm @ v^T                 # (hc, hc)
    out[b,h] = context^T @ query_sm         # (hc, n)

Sharding: data-parallel over batch B=8 across the 8 NeuronCores (no
collectives).  Inputs are ~N(0,1) so softmax needs no max-subtraction.

Layout: the host pre-transposes BOTH inputs into an "A-layout"
[128, N] bf16 array (partition p holds n = j*128+p for every
channel-block j), so every DMA is a plain contiguous slice.  x1 gets a
ones-column per block (129 channels) so the context matmul emits
rowsums for free.  Total HBM traffic 12 MiB/core (8 in + 4 out) —
the kernel is structured to be bound by exactly that.

Single merged pipeline per slab of n (no phase barrier):
    eT   = exp(x2T)                                    (scalar)
    ctx += eT_j^T @ [vT_j | 1]                         (PE, early: only
                                                        needs eT, so bd is
                                                        ready right after
                                                        the last input)
    csT  = per-head colsums: pairwise tree             (gpsimd 8-wide step,
                                                        vector small steps)
    qsmT = eT * bcast(1/csT)    (pair-packed, DVE 2x)  (vector)
    qsm  = PE-transpose(qsmT), lagged 2 slabs          (PE; PSUM->SBUF
                                                        copies on vector)
Then bd = blockdiag(ctx * 1/rowsum) and the attended tiles
    out[:, t] = bd^T @ qsm[:, t]                       (PE + casts + DMA)
stream out per-tile as soon as bd and their qsm window exist.
"""

import numpy as np
from contextlib import ExitStack

B, C, H, W = 8, 128, 128, 128
N = H * W                 # 16384
J = N // 128              # 128 channel-blocks
HEADS, HC = 8, 16
NCORES = 8

SLABS = [2048] * 7 + [1024, 512, 512]
assert sum(SLABS) == N
LAG = 2                   # transpose lag (slabs) so the PE never stalls
QS = 512                  # tail matmul moving-operand tile (one PSUM bank f32)
OT = 2048                 # tail output tile width
NOT = N // OT             # 8

_cache: dict = {}


def _build():
    import concourse.bass as bass
    import concourse.tile as tile
    from concourse import bacc, mybir

    FP32 = mybir.dt.float32
    BF16 = mybir.dt.bfloat16
    AF = mybir.ActivationFunctionType

    nc = bacc.Bacc("TRN2", target_bir_lowering=False, debug=False)

    x2p = nc.dram_tensor("x2p", [128, N], BF16, kind="ExternalInput")
    x1p = nc.dram_tensor("x1p", [128, J * 129], BF16, kind="ExternalInput")
    bd8_in = nc.dram_tensor("bd8", [C, C], BF16, kind="ExternalInput")
    ident_in = nc.dram_tensor("ident", [C, C], BF16, kind="ExternalInput")
    out = nc.dram_tensor("out", [C, N], BF16, kind="ExternalOutput")

    with tile.TileContext(nc) as tc:
        with ExitStack() as ctx:
            persist = ctx.enter_context(tc.tile_pool(name="persist", bufs=1))
            x2ld = ctx.enter_context(tc.tile_pool(name="x2ld", bufs=4))
            vld = ctx.enter_context(tc.tile_pool(name="vld", bufs=4))
            eTp = ctx.enter_context(tc.tile_pool(name="eTp", bufs=4))
            qsp = ctx.enter_context(tc.tile_pool(name="qsp", bufs=4))
            nrm = ctx.enter_context(tc.tile_pool(name="nrm", bufs=3))
            outp = ctx.enter_context(tc.tile_pool(name="outp", bufs=3))
            smalls = ctx.enter_context(tc.tile_pool(name="smalls", bufs=1))

            qsm_nat = persist.tile([C, N], BF16, tag="qsm_nat")
            bd8 = smalls.tile([C, C], BF16, tag="bd8")
            ident = smalls.tile([C, C], BF16, tag="ident")

            with tc.tile_pool(name="pstr", bufs=3, space="PSUM") as ps_tr, \
                 tc.tile_pool(name="psctx", bufs=1, space="PSUM") as ps_ctx:
                ctx_ps = ps_ctx.tile([C, 129], FP32, tag="ctx")

                mm_idx = 0
                trq = []          # queued (qsmT, off, SW) awaiting transpose

                def emit_transpose(qsmT, toff, SW, copy_eng):
                    te = ps_tr.tile([C, SW], BF16, tag="te")
                    for j in range(SW // 128):
                        nc.tensor.transpose(
                            te[:, bass.ts(j, 128)],
                            qsmT[:, bass.ds(j * 128, 128)],
                            ident[:],
                        )
                    go = bass.ds(toff, SW)
                    if copy_eng == "v":
                        nc.vector.tensor_copy(qsm_nat[:, go], te[:])
                    else:
                        nc.scalar.copy(qsm_nat[:, go], te[:])

                off = 0
                for i, SW in enumerate(SLABS):
                    Ji = SW // 128
                    G = Ji * 8
                    x2t = x2ld.tile([128, SW], BF16, tag="x2t")
                    nc.sync.dma_start(out=x2t[:], in_=x2p[:, bass.ds(off, SW)])
                    vT = vld.tile([128, Ji * 129], BF16, tag="vT")
                    nc.sync.dma_start(
                        out=vT[:],
                        in_=x1p[:, bass.ds((off // 128) * 129, Ji * 129)],
                    )
                    if i == 0:
                        nc.scalar.dma_start(out=ident[:], in_=ident_in[:])
                        nc.scalar.dma_start(out=bd8[:], in_=bd8_in[:])

                    eT = eTp.tile([128, SW], BF16, tag="eT")
                    nc.scalar.activation(eT[:], x2t[:], AF.Exp)

                    # context accumulation: current slab, ahead of any
                    # transpose in PE order — its inputs are ready early.
                    for j in range(Ji):
                        nc.tensor.matmul(
                            ctx_ps[:],
                            eT[:, bass.ts(j, 128)],
                            vT[:, j * 129:(j + 1) * 129],
                            start=(mm_idx == 0),
                            stop=(mm_idx == J - 1),
                        )
                        mm_idx += 1

                    # per-head colsums: two pairwise tree steps on gpsimd
                    # (16->8->4), one 4-wide reduce on vector. Minimizes
                    # vector instruction count (per-instruction overhead is
                    # the dominant cost, not element throughput).
                    e4 = eT[:].rearrange("p (g c) -> p g c", c=16)
                    t1 = nrm.tile([128, G * 8], BF16, tag="t1")
                    with nc.allow_low_precision("bf16 pairwise colsum; tol 2e-2"):
                        nc.gpsimd.tensor_add(
                            t1[:].rearrange("p (g c) -> p g c", c=8),
                            e4[:, :, 0:8], e4[:, :, 8:16],
                        )
                        t2 = nrm.tile([128, G * 4], BF16, tag="t2")
                        t1v = t1[:].rearrange("p (g c) -> p g c", c=8)
                        nc.gpsimd.tensor_add(
                            t2[:].rearrange("p (g c) -> p g c", c=4),
                            t1v[:, :, 0:4], t1v[:, :, 4:8],
                        )
                        csf = nrm.tile([128, G], FP32, tag="csf")
                        nc.vector.tensor_reduce(
                            csf[:],
                            t2[:].rearrange("p (g c) -> p g c", c=4),
                            mybir.AxisListType.X, mybir.AluOpType.add,
                        )
                    rcf = nrm.tile([128, G], FP32, tag="rcf")
                    nc.vector.reciprocal_approx_fast(out=rcf[:], in_=csf[:])
                    # duplicate reciprocals into packed bf16 pairs so the qsm
                    # multiply keeps an innermost unit stride (DVE 2x mode);
                    # the tiny strided cast runs on the scalar engine.
                    rc2 = nrm.tile([128, G * 2], BF16, tag="rc2")
                    nc.scalar.copy(
                        rc2[:].rearrange("p (g t) -> p g t", t=2),
                        rcf[:, :, None].broadcast_to([128, G, 2]),
                    )
                    qsmT = qsp.tile([128, SW], BF16, tag="qsmT")
                    nc.vector.tensor_mul(
                        qsmT[:].rearrange("p (g a t) -> p g a t", a=8, t=2),
                        eT[:].rearrange("p (g a t) -> p g a t", a=8, t=2),
                        rc2[:].rearrange("p (g t) -> p g t", t=2)[:, :, None, :]
                              .broadcast_to([128, G, 8, 2]),
                    )
                    trq.append((qsmT, off, SW))
                    if len(trq) > LAG:
                        emit_transpose(*trq.pop(0), copy_eng="v" if i < 7 else "s")
                    off += SW
                for k, args in enumerate(trq):
                    emit_transpose(*args, copy_eng="s")

                # ---- block-diagonal context weights ----
                rs_rcp = smalls.tile([C, 1], FP32, tag="rs_rcp")
                nc.vector.reciprocal(rs_rcp[:], ctx_ps[:, 128:129])
                scaled = smalls.tile([C, C], BF16, tag="scaled")
                nc.vector.tensor_scalar(
                    scaled[:], ctx_ps[:, 0:128], rs_rcp[:, 0:1], None,
                    mybir.AluOpType.mult,
                )
                bd = smalls.tile([C, C], BF16, tag="bd")
                nc.vector.tensor_mul(bd[:], scaled[:], bd8[:])

            # ---- attended tiles: matmul, cast, store ----
            with tc.tile_pool(name="psatt", bufs=2, space="PSUM") as ps_att:
                for t in range(NOT):
                    att = ps_att.tile([C, OT], FP32, tag="att")
                    for q in range(OT // QS):
                        nc.tensor.matmul(
                            att[:, bass.ts(q, QS)], bd[:],
                            qsm_nat[:, bass.ds(t * OT + q * QS, QS)],
                        )
                    ot = outp.tile([C, OT], BF16, tag="ot")
                    if t in (1, 4, 6):
                        nc.vector.tensor_copy(ot[:], att[:])
                    else:
                        nc.scalar.copy(ot[:], att[:])
                    nc.scalar.dma_start(out=out[:, bass.ts(t, OT)], in_=ot[:])

    nc.compile()
    return nc


def _get_nc():
    if "nc" not in _cache:
        _cache["nc"] = _build()
    return _cache["nc"]


def _bd8_np() -> np.ndarray:
    import ml_dtypes

    m = np.zeros((C, C), dtype=np.float32)
    for h in range(HEADS):
        m[h * HC:(h + 1) * HC, h * HC:(h + 1) * HC] = 1.0
    return m.astype(ml_dtypes.bfloat16)


def _ident_np() -> np.ndarray:
    import ml_dtypes

    return np.eye(C, dtype=np.float32).astype(ml_dtypes.bfloat16)


def _to_np(a) -> np.ndarray:
    """Materialize to float32 numpy; retry once on a transient bad fetch
    (device-backed arrays have been observed to materialize NaNs once)."""
    out = np.asarray(a, dtype=np.float32)
    if np.isnan(out).any():
        out = np.asarray(a, dtype=np.float32)
    return out


def _in_maps(x1: np.ndarray, x2: np.ndarray) -> list[dict]:
    """Host-side sharding + layout: per-core A-layout bf16 arrays."""
    import ml_dtypes

    BF = ml_dtypes.bfloat16
    x1 = _to_np(x1).reshape(B, C, N)
    x2 = _to_np(x2).reshape(B, C, N)
    # A-layout: arr[b, p, j, c] = x[b, c, j*128 + p]
    x2a = np.ascontiguousarray(
        x2.reshape(B, C, J, 128).transpose(0, 3, 2, 1)).astype(BF)
    x1a = x1.reshape(B, C, J, 128).transpose(0, 3, 2, 1).astype(BF)
    x1e = np.ones((B, 128, J, 129), dtype=BF)
    x1e[..., :128] = x1a
    x2a = x2a.reshape(B, 128, N)
    x1e = x1e.reshape(B, 128, J * 129)
    bd8 = _bd8_np()
    ident = _ident_np()
    return [
        {"x2p": x2a[i], "x1p": x1e[i], "bd8": bd8, "ident": ident}
        for i in range(NCORES)
    ]


def kernel(x1: np.ndarray, x2: np.ndarray) -> np.ndarray:
    from concourse.bass_utils import run_bass_kernel_spmd

    nc = _get_nc()
    in_maps = _in_maps(x1, x2)
    res = run_bass_kernel_spmd(nc, in_maps, core_ids=list(range(NCORES)))
    outs = [np.asarray(res.results[i]["out"], dtype=np.float32) for i in range(NCORES)]
    return np.stack(outs, axis=0).reshape(B, C, H, W)
